# revision 8
# baseline (speedup 1.0000x reference)
"""Trainium2 Bass kernel for DfaRnn forward: out[b,t] = tanh(x_t @ W_xh + h_{t-1} @ W_hh + b).

Strategy (8 NeuronCores, data-parallel over batch, B=16 -> 2 rows/core):
  - Host pre-transposes inputs so the device kernel needs no transposes:
      xt[p, dc, t, b]  = x[b, t, dc*128+p]          (fp16)
      wxh[p, dc, mc, j] = W_xh[dc*128+p, mc*128+j]  (fp16)
      whh[p, kc, mc, j] = W_hh[kc*128+p, mc*128+j]  (fp16)
      bt[p, mc]        = b[mc*128+p]                (fp32)
  - Phase 1 (parallel): xp = x @ W_xh + b, computed as 512-col matmul groups,
    PE -> PSUM -> DVE(+bias) -> SBUF fp32.
  - Phase 2 (sequential, 2048 steps): h_t kept as hT [4x128 partitions, 2 cols].
    Per step: 16 matmuls (W_hh tiles stationary, h columns moving) accumulate
    W_hh^T-oriented product into 8 PSUM banks (4 m-chunks x 2 step-parity),
    DVE adds xp_t, ACT applies tanh -> hs_sb fp16 (which doubles as the rhs
    history for the next step and the output staging buffer).
    The step is split in halves (m-chunks 01 / 23, k-chunks 01 / 23) so the
    DVE/ACT tail of step t overlaps the PE sweep of step t+1.
  - Output hs[p, mc, t, b] fp16 DMA'd out; host reassembles [B, T, H] fp32.

Numerics: fp16 storage for W/x/h with fp32 PSUM accumulation. Measured on CPU
emulation: global rel err ~4.5e-4 vs fp64 reference (errors are contractive,
not accumulating). Output returned as fp32.
"""

import os
import sys

import numpy as np

for _p in ("/opt/trn_rl_repo",):
    if os.path.isdir(_p) and _p not in sys.path:
        sys.path.append(_p)

import concourse.bass as bass
import concourse.mybir as mybir
from concourse import bass_utils

P = 128          # partitions
H = 512          # hidden dim
D = 512          # input dim
NCH = H // P     # 4 h-chunks
NCD = D // P     # 4 d-chunks
N_CORES = 8

f16 = mybir.dt.float16
f32 = mybir.dt.float32


def build_nc(T: int, b_local: int, strict_sync: bool = False):
    """Build the per-core Bass program (SPMD; same program on all cores).

    strict_sync adds always-satisfied waits that CoreSim's race checker
    demands (it doesn't credit transitive or same-engine ordering when the
    writer carries a sem update). Real hardware respects engine program order
    and transitive completion, so the HW build omits them to save issue slots.
    """
    cols = T * b_local
    gcols = 512 if cols % 512 == 0 else cols     # xproj matmul group width
    assert cols % gcols == 0
    G = cols // gcols                            # xproj groups
    spg = gcols // b_local                       # timesteps per xproj group

    nc = bass.Bass("TRN2", target_bir_lowering=False, debug=False)

    # DRAM I/O
    xt_d = nc.dram_tensor("xt", [P, NCD, T, b_local], f16, kind="ExternalInput")
    wxh_d = nc.dram_tensor("wxh", [P, NCD, NCH, P], f16, kind="ExternalInput")
    whh_d = nc.dram_tensor("whh", [P, NCH, NCH, P], f16, kind="ExternalInput")
    bt_d = nc.dram_tensor("bt", [P, NCH], f32, kind="ExternalInput")
    hs_d = nc.dram_tensor("hs", [P, NCH, T, b_local], f16, kind="ExternalOutput")

    # SBUF
    xt = nc.alloc_sbuf_tensor("xt_sb", [P, NCD, T, b_local], f16)
    wxh = nc.alloc_sbuf_tensor("wxh_sb", [P, NCD, NCH, P], f16)
    whh = nc.alloc_sbuf_tensor("whh_sb", [P, NCH, NCH, P], f16)
    bt = nc.alloc_sbuf_tensor("bt_sb", [P, NCH], f32)
    xp = nc.alloc_sbuf_tensor("xp_sb", [P, NCH, T, b_local], f32)
    hs = nc.alloc_sbuf_tensor("hs_sb", [P, NCH, T, b_local], f16)
    z = nc.alloc_sbuf_tensor("z_sb", [P, 2, NCH, b_local], f32)

    # PSUM: 8 banks of [128, 512] f32. Bank(2*mc + parity) for the recurrence
    # ([:, :, 0:b_local]); xproj reuses the same banks (full width) earlier.
    ps = nc.alloc_psum_tensor("ps", [P, 8, 512], f32)

    in_sem = nc.alloc_semaphore("in_sem")
    pex_sem = nc.alloc_semaphore("pex_sem")
    dvex_sem = nc.alloc_semaphore("dvex_sem")
    pe_sem = nc.alloc_semaphore("pe_sem")
    dve_sem = nc.alloc_semaphore("dve_sem")
    act_sem = nc.alloc_semaphore("act_sem")
    out_sem = nc.alloc_semaphore("out_sem")

    Tanh = mybir.ActivationFunctionType.Tanh

    with nc.Block() as block:

        @block.sync
        def _(sync):
            sync.dma_start(xt.ap(), xt_d.ap()).then_inc(in_sem, 16)
            sync.dma_start(wxh.ap(), wxh_d.ap()).then_inc(in_sem, 16)
            sync.dma_start(whh.ap(), whh_d.ap()).then_inc(in_sem, 16)
            sync.dma_start(bt.ap(), bt_d.ap()).then_inc(in_sem, 16)
            sync.wait_ge(act_sem, 2 * T)
            sync.dma_start(hs_d.ap(), hs.ap()).then_inc(out_sem, 16)
            sync.wait_ge(out_sem, 16)

        @block.tensor
        def _(tensor):
            tensor.wait_ge(in_sem, 64)
            # -- Phase 1: xproj --
            for g in range(G):
                for mc in range(NCH):
                    bank = 2 * mc + (g % 2)
                    if g >= 2:
                        tensor.wait_ge(dvex_sem, NCH * (g - 2) + mc + 1)
                    mm = None
                    for dc in range(NCD):
                        mm = tensor.matmul(
                            ps[:, bank, 0:gcols],
                            wxh[:, dc, mc, :],
                            xt[:, dc, g * spg:(g + 1) * spg, :],
                            start=(dc == 0),
                            stop=(dc == NCD - 1),
                        )
                    mm.then_inc(pex_sem, 1)
            # -- Phase 2: recurrence sweeps --
            for t in range(1, T):
                par = t % 2
                if strict_sync and t >= 2:
                    # psum parity banks were read by DVE of step t-2; implied
                    # by the act_sem wait below (transitively); the race
                    # checker needs it spelled out.
                    tensor.wait_ge(dve_sem, 2 * (t - 2) + 2)
                def mm(mc, kc):
                    return tensor.matmul(
                        ps[:, 2 * mc + par, 0:b_local],
                        whh[:, kc, mc, :],
                        hs[:, kc, t - 1, :],
                        start=(kc == 0),
                        stop=(kc == 3),
                        skip_group_check=True,
                    )

                # Steady-state-optimal order (cycle = max(k_h1, 16-c_h2)*r + L):
                # 6 kc01-matmuls, wait for tanh h2 of t-1, finish groups 0,1
                # (releases tanh h1 inputs at MM 10), then the rest.
                tensor.wait_ge(act_sem, 2 * t - 1)
                for mc, kc in ((0, 0), (0, 1), (1, 0), (1, 1), (2, 0), (2, 1)):
                    mm(mc, kc)
                tensor.wait_ge(act_sem, 2 * t)
                for mc, kc in ((0, 2), (0, 3), (1, 2), (1, 3)):
                    m = mm(mc, kc)
                m.then_inc(pe_sem, 1)
                for mc, kc in ((3, 0), (3, 1), (2, 2), (2, 3), (3, 2), (3, 3)):
                    m = mm(mc, kc)
                m.then_inc(pe_sem, 1)

        @block.vector
        def _(vector):
            vector.wait_ge(in_sem, 64)  # bt read below (race-checker explicit)
            # -- Phase 1: PSUM -> xp copies (+ bias) --
            for g in range(G):
                for mc in range(NCH):
                    bank = 2 * mc + (g % 2)
                    vector.wait_ge(pex_sem, NCH * g + mc + 1)
                    vector.tensor_scalar_add(
                        xp[:, mc, g * spg:(g + 1) * spg, :],
                        ps[:, bank, 0:gcols],
                        bt[:, mc:mc + 1],
                    ).then_inc(dvex_sem, 1)
            # -- Phase 2: z = psum + xp --
            if strict_sync:
                # own-engine xp writes are program-ordered; explicit for checker
                vector.wait_ge(dvex_sem, NCH * G)
            # step 0: z = xp[0] (h_{-1} = 0, no matmul)
            for half in (0, 1):
                vector.tensor_copy(
                    z[:, 0, 2 * half:2 * half + 2, :],
                    xp[:, 2 * half:2 * half + 2, 0, :],
                ).then_inc(dve_sem, 1)
            for t in range(1, T):
                par = t % 2
                if strict_sync and t >= 2:
                    # z[par] was read by ACT of step t-2 (implied; explicit
                    # for the race checker, never stalls)
                    vector.wait_ge(act_sem, 2 * (t - 2) + 2)
                for half in (0, 1):
                    vector.wait_ge(pe_sem, 2 * (t - 1) + half + 1)
                    # psum banks {4*half + par, 4*half + 2 + par}, cols 0:b_local
                    src = bass.AP(
                        ps,
                        (4 * half + par) * 512,
                        [[8 * 512, P], [2 * 512, 2], [1, b_local]],
                    )
                    vector.tensor_add(
                        z[:, par, 2 * half:2 * half + 2, :],
                        src,
                        xp[:, 2 * half:2 * half + 2, t, :],
                    ).then_inc(dve_sem, 1)

        @block.scalar
        def _(scalar):
            for t in range(T):
                par = t % 2
                for half in (0, 1):
                    scalar.wait_ge(dve_sem, 2 * t + half + 1)
                    scalar.activation(
                        hs[:, 2 * half:2 * half + 2, t, :],
                        z[:, par, 2 * half:2 * half + 2, :],
                        Tanh,
                    ).then_inc(act_sem, 1)

    return nc


def prep_inputs(x, W_xh, W_hh, b, b_local):
    """Host-side layout transforms. Returns (shared, per_core_list)."""
    T = x.shape[1]
    wxh_np = np.ascontiguousarray(
        W_xh.reshape(NCD, P, NCH, P).transpose(1, 0, 2, 3)).astype(np.float16)
    whh_np = np.ascontiguousarray(
        W_hh.reshape(NCH, P, NCH, P).transpose(1, 0, 2, 3)).astype(np.float16)
    bt_np = np.ascontiguousarray(b.reshape(NCH, P).T).astype(np.float32)
    in_maps = []
    for c in range(N_CORES):
        xc = x[c * b_local:(c + 1) * b_local]  # [b_local, T, D]
        xt_np = np.ascontiguousarray(
            xc.transpose(2, 1, 0).reshape(NCD, P, T, b_local).transpose(1, 0, 2, 3)
        ).astype(np.float16)
        in_maps.append({"xt": xt_np, "wxh": wxh_np, "whh": whh_np, "bt": bt_np})
    return in_maps


def assemble_output(core_outs, T, b_local):
    B = N_CORES * b_local
    full = np.empty((B, T, H), np.float32)
    for c in range(N_CORES):
        hs_np = core_outs[c]["hs"]  # [P, NCH, T, b_local] fp16
        full[c * b_local:(c + 1) * b_local] = (
            hs_np.transpose(3, 2, 1, 0).reshape(b_local, T, H).astype(np.float32))
    return full


_NC_CACHE = {}


def _get_nc(T, b_local):
    key = (T, b_local)
    if key not in _NC_CACHE:
        _NC_CACHE[key] = build_nc(T, b_local)
    return _NC_CACHE[key]


def run_on_device(inputs, trace=False, **spmd_kwargs):
    x = np.asarray(inputs["x"], np.float32)
    W_xh = np.asarray(inputs["W_xh"], np.float32)
    W_hh = np.asarray(inputs["W_hh"], np.float32)
    b = np.asarray(inputs["b"], np.float32)
    # A affects only the backward pass; the forward output does not use it.
    B, T, D_ = x.shape
    assert D_ == D and W_xh.shape == (D, H) and W_hh.shape == (H, H)
    assert B % N_CORES == 0
    b_local = B // N_CORES

    nc = _get_nc(T, b_local)
    in_maps = prep_inputs(x, W_xh, W_hh, b, b_local)
    res = bass_utils.run_bass_kernel_spmd(
        nc, in_maps, core_ids=list(range(N_CORES)), trace=trace, **spmd_kwargs)
    return assemble_output(res.results, T, b_local), res


def kernel(**inputs):
    out, _ = run_on_device(inputs)
    return out


# revision 13
# speedup vs baseline: 1.9530x; 1.9530x over previous
"""Trainium2 Bass kernel for DfaRnn forward: out[b,t] = tanh(x_t @ W_xh + h_{t-1} @ W_hh + b).

Strategy (8 NeuronCores, data-parallel over batch, B=16 -> 2 rows/core):
  - Host pre-transposes inputs so the device kernel needs no transposes:
      xt[p, dc, t, b]  = x[b, t, dc*128+p]          (fp16)
      wxh[p, dc, mc, j] = W_xh[dc*128+p, mc*128+j]  (fp16)
      whh[p, kc, mc, j] = W_hh[kc*128+p, mc*128+j]  (fp16)
      bt[p, mc]        = b[mc*128+p]                (fp32)
  - Phase 1 (parallel): xp = x @ W_xh + b, computed as 512-col matmul groups,
    PE -> PSUM -> DVE(+bias) -> SBUF fp32.
  - Phase 2 (sequential, 2048 steps): h_t kept as hT [4x128 partitions, 2 cols].
    Per step: 16 matmuls (W_hh tiles stationary, h columns moving) accumulate
    W_hh^T-oriented product into 8 PSUM banks (4 m-chunks x 2 step-parity),
    DVE adds xp_t, ACT applies tanh -> hs_sb fp16 (which doubles as the rhs
    history for the next step and the output staging buffer).
    The step is split in halves (m-chunks 01 / 23, k-chunks 01 / 23) so the
    DVE/ACT tail of step t overlaps the PE sweep of step t+1.
  - Output hs[p, mc, t, b] fp16 DMA'd out; host reassembles [B, T, H] fp32.

Numerics: fp16 storage for W/x/h with fp32 PSUM accumulation. Measured on CPU
emulation: global rel err ~4.5e-4 vs fp64 reference (errors are contractive,
not accumulating). Output returned as fp32.
"""

import os
import sys

import numpy as np

for _p in ("/opt/trn_rl_repo",):
    if os.path.isdir(_p) and _p not in sys.path:
        sys.path.append(_p)

import concourse.bass as bass
import concourse.mybir as mybir
from concourse import bass_utils

P = 128          # partitions
H = 512          # hidden dim
D = 512          # input dim
NCH = H // P     # 4 h-chunks
NCD = D // P     # 4 d-chunks
N_CORES = 8

f16 = mybir.dt.float16
f32 = mybir.dt.float32


PROBE_NP = 160          # probes
PROBE_PERIOD_CYC = 60000  # gpsimd NX cycles between probes (~50us @ 0.8333ns/cyc)
PROBE_MARKS = 32        # timesteps sampled per probe (t = k*T/32)


def build_nc(T: int, b_local: int, strict_sync: bool = False, probe: bool = False):
    """Build the per-core Bass program (SPMD; same program on all cores).

    strict_sync adds always-satisfied waits that CoreSim's race checker
    demands (it doesn't credit transitive or same-engine ordering when the
    writer carries a sem update). Real hardware respects engine program order
    and transitive completion, so the HW build omits them to save issue slots.
    """
    cols = T * b_local
    gcols = 512 if cols % 512 == 0 else cols     # xproj matmul group width
    assert cols % gcols == 0
    G = cols // gcols                            # xproj groups
    spg = gcols // b_local                       # timesteps per xproj group

    nc = bass.Bass("TRN2", target_bir_lowering=False, debug=False)

    # DRAM I/O
    xt_d = nc.dram_tensor("xt", [P, NCD, T, b_local], f16, kind="ExternalInput")
    wxh_d = nc.dram_tensor("wxh", [P, NCD, NCH, P], f16, kind="ExternalInput")
    whh_d = nc.dram_tensor("whh", [P, NCH, NCH, P], f16, kind="ExternalInput")
    bt_d = nc.dram_tensor("bt", [P, NCH], f32, kind="ExternalInput")
    hs_d = nc.dram_tensor("hs", [P, NCH, T, b_local], f16, kind="ExternalOutput")

    # SBUF
    xt = nc.alloc_sbuf_tensor("xt_sb", [P, NCD, T, b_local], f16)
    wxh = nc.alloc_sbuf_tensor("wxh_sb", [P, NCD, NCH, P], f16)
    whh = nc.alloc_sbuf_tensor("whh_sb", [P, NCH, NCH, P], f16)
    bt = nc.alloc_sbuf_tensor("bt_sb", [P, NCH], f32)
    xp = nc.alloc_sbuf_tensor("xp_sb", [P, NCH, T, b_local], f32)
    hs = nc.alloc_sbuf_tensor("hs_sb", [P, NCH, T, b_local], f16)
    z = nc.alloc_sbuf_tensor("z_sb", [P, 2, NCH, b_local], f32)

    # PSUM: 8 banks of [128, 512] f32. Bank(2*mc + parity) for the recurrence
    # ([:, :, 0:b_local]); xproj reuses the same banks (full width) earlier.
    ps = nc.alloc_psum_tensor("ps", [P, 8, 512], f32)

    if probe:
        probes_d = nc.dram_tensor(
            "probes", [1, PROBE_NP, PROBE_MARKS], f16, kind="ExternalOutput")
        probes = nc.alloc_sbuf_tensor("probes_sb", [1, PROBE_NP, PROBE_MARKS], f16)
        gz_sem = nc.alloc_semaphore("gz_sem")
        gp_sem = nc.alloc_semaphore("gp_sem")

    in_sem = nc.alloc_semaphore("in_sem")
    pex_sem = nc.alloc_semaphore("pex_sem")
    dvex_sem = nc.alloc_semaphore("dvex_sem")
    pe_sem = nc.alloc_semaphore("pe_sem")
    dve_sem = nc.alloc_semaphore("dve_sem")
    act_sem = nc.alloc_semaphore("act_sem")
    out_sem = nc.alloc_semaphore("out_sem")

    Tanh = mybir.ActivationFunctionType.Tanh

    with nc.Block() as block:

        @block.sync
        def _(sync):
            sync.dma_start(xt.ap(), xt_d.ap()).then_inc(in_sem, 16)
            sync.dma_start(wxh.ap(), wxh_d.ap()).then_inc(in_sem, 16)
            sync.dma_start(whh.ap(), whh_d.ap()).then_inc(in_sem, 16)
            sync.dma_start(bt.ap(), bt_d.ap()).then_inc(in_sem, 16)
            sync.wait_ge(act_sem, 2 * T)
            sync.dma_start(hs_d.ap(), hs.ap()).then_inc(out_sem, 16)
            if probe:
                sync.wait_ge(gp_sem, 1)
                sync.dma_start(probes_d.ap(), probes.ap()).then_inc(out_sem, 16)
                sync.wait_ge(out_sem, 32)
            else:
                sync.wait_ge(out_sem, 16)

        @block.tensor
        def _(tensor):
            tensor.wait_ge(in_sem, 64)
            # -- Phase 1: xproj --
            for g in range(G):
                for mc in range(NCH):
                    bank = 2 * mc + (g % 2)
                    if g >= 2:
                        tensor.wait_ge(dvex_sem, NCH * (g - 2) + mc + 1)
                    mm = None
                    for dc in range(NCD):
                        mm = tensor.matmul(
                            ps[:, bank, 0:gcols],
                            wxh[:, dc, mc, :],
                            xt[:, dc, g * spg:(g + 1) * spg, :],
                            start=(dc == 0),
                            stop=(dc == NCD - 1),
                        )
                    mm.then_inc(pex_sem, 1)
            # -- Phase 2: recurrence sweeps --
            for t in range(1, T):
                par = t % 2
                if strict_sync and t >= 2:
                    # psum parity banks were read by DVE of step t-2; implied
                    # by the act_sem wait below (transitively); the race
                    # checker needs it spelled out.
                    tensor.wait_ge(dve_sem, 2 * (t - 2) + 2)
                def mm(mc, kc):
                    return tensor.matmul(
                        ps[:, 2 * mc + par, 0:b_local],
                        whh[:, kc, mc, :],
                        hs[:, kc, t - 1, :],
                        start=(kc == 0),
                        stop=(kc == 3),
                        skip_group_check=True,
                    )

                # Steady-state-optimal order (cycle = max(k_h1, 16-c_h2)*r + L):
                # 6 kc01-matmuls, wait for tanh h2 of t-1, finish groups 0,1
                # (releases tanh h1 inputs at MM 10), then the rest.
                tensor.wait_ge(act_sem, 2 * t - 1)
                for mc, kc in ((0, 0), (0, 1), (1, 0), (1, 1), (2, 0), (2, 1)):
                    mm(mc, kc)
                tensor.wait_ge(act_sem, 2 * t)
                for mc, kc in ((0, 2), (0, 3), (1, 2), (1, 3)):
                    m = mm(mc, kc)
                m.then_inc(pe_sem, 1)
                for mc, kc in ((3, 0), (3, 1), (2, 2), (2, 3), (3, 2), (3, 3)):
                    m = mm(mc, kc)
                m.then_inc(pe_sem, 1)

        @block.vector
        def _(vector):
            vector.wait_ge(in_sem, 64)  # bt read below (race-checker explicit)
            # -- Phase 1: PSUM -> xp copies (+ bias) --
            for g in range(G):
                for mc in range(NCH):
                    bank = 2 * mc + (g % 2)
                    vector.wait_ge(pex_sem, NCH * g + mc + 1)
                    vector.tensor_scalar_add(
                        xp[:, mc, g * spg:(g + 1) * spg, :],
                        ps[:, bank, 0:gcols],
                        bt[:, mc:mc + 1],
                    ).then_inc(dvex_sem, 1)
            # -- Phase 2: z = psum + xp --
            if strict_sync:
                # own-engine xp writes are program-ordered; explicit for checker
                vector.wait_ge(dvex_sem, NCH * G)
            # step 0: z = xp[0] (h_{-1} = 0, no matmul)
            for half in (0, 1):
                vector.tensor_copy(
                    z[:, 0, 2 * half:2 * half + 2, :],
                    xp[:, 2 * half:2 * half + 2, 0, :],
                ).then_inc(dve_sem, 1)
            for t in range(1, T):
                par = t % 2
                if strict_sync and t >= 2:
                    # z[par] was read by ACT of step t-2 (implied; explicit
                    # for the race checker, never stalls)
                    vector.wait_ge(act_sem, 2 * (t - 2) + 2)
                for half in (0, 1):
                    vector.wait_ge(pe_sem, 2 * (t - 1) + half + 1)
                    # psum banks {4*half + par, 4*half + 2 + par}, cols 0:b_local
                    src = bass.AP(
                        ps,
                        (4 * half + par) * 512,
                        [[8 * 512, P], [2 * 512, 2], [1, b_local]],
                    )
                    vector.tensor_add(
                        z[:, par, 2 * half:2 * half + 2, :],
                        src,
                        xp[:, 2 * half:2 * half + 2, t, :],
                    ).then_inc(dve_sem, 1)

        if probe:
            # GPSIMD (otherwise idle) = on-device profiler: zero hs, then
            # snapshot hs completion cells every ~20us. tanh outputs are
            # generically nonzero, so snapshot != 0 <=> step done.
            @block.gpsimd
            def _(gpsimd):
                gpsimd.memset(hs.ap(), 0).then_inc(gz_sem, 1)
                pitch = NCH * T * b_local
                snap = bass.AP(
                    hs, 3 * T * b_local,
                    [[pitch, 1], [(T // PROBE_MARKS) * b_local, PROBE_MARKS]])
                cp = None
                for i in range(PROBE_NP):
                    gpsimd.nop(cycle_cnt=PROBE_PERIOD_CYC)
                    cp = gpsimd.tensor_copy(probes[0:1, i, :], snap)
                cp.then_inc(gp_sem, 1)

        @block.scalar
        def _(scalar):
            if probe:
                scalar.wait_ge(gz_sem, 1)
            for t in range(T):
                par = t % 2
                for half in (0, 1):
                    scalar.wait_ge(dve_sem, 2 * t + half + 1)
                    scalar.activation(
                        hs[:, 2 * half:2 * half + 2, t, :],
                        z[:, par, 2 * half:2 * half + 2, :],
                        Tanh,
                    ).then_inc(act_sem, 1)

    return nc


def prep_inputs(x, W_xh, W_hh, b, b_local):
    """Host-side layout transforms. Returns (shared, per_core_list)."""
    T = x.shape[1]
    wxh_np = np.ascontiguousarray(
        W_xh.reshape(NCD, P, NCH, P).transpose(1, 0, 2, 3)).astype(np.float16)
    whh_np = np.ascontiguousarray(
        W_hh.reshape(NCH, P, NCH, P).transpose(1, 0, 2, 3)).astype(np.float16)
    bt_np = np.ascontiguousarray(b.reshape(NCH, P).T).astype(np.float32)
    in_maps = []
    for c in range(N_CORES):
        xc = x[c * b_local:(c + 1) * b_local]  # [b_local, T, D]
        xt_np = np.ascontiguousarray(
            xc.transpose(2, 1, 0).reshape(NCD, P, T, b_local).transpose(1, 0, 2, 3)
        ).astype(np.float16)
        in_maps.append({"xt": xt_np, "wxh": wxh_np, "whh": whh_np, "bt": bt_np})
    return in_maps


def assemble_output(core_outs, T, b_local):
    B = N_CORES * b_local
    full = np.empty((B, T, H), np.float32)
    for c in range(N_CORES):
        hs_np = core_outs[c]["hs"]  # [P, NCH, T, b_local] fp16
        full[c * b_local:(c + 1) * b_local] = (
            hs_np.transpose(3, 2, 1, 0).reshape(b_local, T, H).astype(np.float32))
    return full


_NC_CACHE = {}


def _get_nc(T, b_local):
    key = (T, b_local)
    if key not in _NC_CACHE:
        _NC_CACHE[key] = build_nc(T, b_local)
    return _NC_CACHE[key]


def run_on_device(inputs, trace=False, **spmd_kwargs):
    x = np.asarray(inputs["x"], np.float32)
    W_xh = np.asarray(inputs["W_xh"], np.float32)
    W_hh = np.asarray(inputs["W_hh"], np.float32)
    b = np.asarray(inputs["b"], np.float32)
    # A affects only the backward pass; the forward output does not use it.
    B, T, D_ = x.shape
    assert D_ == D and W_xh.shape == (D, H) and W_hh.shape == (H, H)
    assert B % N_CORES == 0
    b_local = B // N_CORES

    nc = _get_nc(T, b_local)
    in_maps = prep_inputs(x, W_xh, W_hh, b, b_local)
    res = bass_utils.run_bass_kernel_spmd(
        nc, in_maps, core_ids=list(range(N_CORES)), trace=trace, **spmd_kwargs)
    return assemble_output(res.results, T, b_local), res


def kernel(**inputs):
    out, _ = run_on_device(inputs)
    return out
